# revision 12
# baseline (speedup 1.0000x reference)
"""Trainium2 Bass kernel for nn_AnotherMamba (selective-scan Mamba block).

Sharding: 8 cores = 2 (batch) x 4 (d_inner chunks of 1024 channels).
Each core runs the full fused pipeline for its (batch, channel-shard):
projections (bf16, TensorE), causal depthwise conv + SiLU, softplus(dt),
selective scan via DVE tensor_tensor_scan, gated output, and a partial
output projection. Host sums the 4 channel-shard partials per batch.
"""
import os
import sys

import numpy as np

sys.path.insert(0, "/opt/trn_rl_repo")

import ml_dtypes  # noqa: E402

BF = ml_dtypes.bfloat16

# Problem dims (hardcoded per harness contract)
B_, L_, DM, DIN, DXB, DS, DC, DTR = 2, 4096, 2048, 4096, 1024, 16, 4, 128
NCORES = 8
NCH = 4                  # d_inner chunk cores per batch
DIN_SH = DIN // NCH      # 1024 channels per core
DXB_SH = DXB // NCH      # 256 B/x rows per core
NCT = DIN_SH // 128      # 8 channel tiles per core

# consts layout (128, NC) f32 columns
_A0 = 0                  # A: cols [0,128): col i*16+s = A[i*128+p, s]
_D0 = 128                # D: 8 cols
_CB0 = 136               # conv_b: 8 cols
_CBN = 144               # -conv_b: 8 cols
_BDT2 = 152              # 2*bdt: 8 cols
_BZ0 = 160               # bz: 8 cols
_BZN = 168               # -bz: 8 cols
_W40 = 176               # conv_w: cols 176 + i*4 + j
_BX0 = 208               # bx in rows 0..31: col 208+i = bx[i*32:(i+1)*32]
_NC = 216


def _build(L, T):
    from concourse import bass, mybir
    from concourse.tile import TileContext

    F32, BF16 = mybir.dt.float32, mybir.dt.bfloat16
    AF = mybir.ActivationFunctionType
    OP = mybir.AluOpType
    NT = L // T
    TH = T // 2
    N4 = T // 128

    nc = bass.Bass()
    hsT = nc.declare_dram_parameter("hsT", [DM, L], BF16, isOutput=False)
    wxbT = nc.declare_dram_parameter("wxbT", [DM, 2 * DXB_SH], BF16, isOutput=False)
    wzT = nc.declare_dram_parameter("wzT", [DM, DIN_SH], BF16, isOutput=False)
    wcT = nc.declare_dram_parameter("wcT", [DM, DIN_SH], BF16, isOutput=False)
    wddT = nc.declare_dram_parameter("wddT", [DM, DTR], BF16, isOutput=False)
    wdtT = nc.declare_dram_parameter("wdtT", [DTR, DIN_SH], BF16, isOutput=False)
    woutT = nc.declare_dram_parameter("woutT", [DIN_SH, DM], BF16, isOutput=False)
    consts = nc.declare_dram_parameter("consts", [128, _NC], F32, isOutput=False)
    e32 = nc.declare_dram_parameter("e32", [32, 128], BF16, isOutput=False)
    ebs = nc.declare_dram_parameter("ebs", [64, 16 * 128], BF16, isOutput=False)
    ecs = nc.declare_dram_parameter("ecs", [128, 16 * 128], BF16, isOutput=False)
    outp = nc.declare_dram_parameter("outp", [DM, L], BF16, isOutput=True)

    with TileContext(nc) as tc:
        with tc.tile_pool(name="wp", bufs=1) as wp, \
             tc.tile_pool(name="hsp", bufs=1) as hsp, \
             tc.tile_pool(name="sp", bufs=2) as sp, \
             tc.tile_pool(name="sp3", bufs=2) as sp3, \
             tc.tile_pool(name="sq", bufs=1) as sq, \
             tc.tile_pool(name="big", bufs=1) as big, \
             tc.tile_pool(name="psA", bufs=3, space="PSUM") as psA, \
             tc.tile_pool(name="psT", bufs=2, space="PSUM") as psT, \
             tc.tile_pool(name="psO", bufs=1, space="PSUM") as psO:

            # ---- resident weights / constants
            w_wxb = wp.tile([128, 16, 2 * DXB_SH], BF16, tag="w_wxb")
            w_wz = wp.tile([128, 16, DIN_SH], BF16, tag="w_wz")
            w_wc = wp.tile([128, 16, DIN_SH], BF16, tag="w_wc")
            w_wdd = wp.tile([128, 16, DTR], BF16, tag="w_wdd")
            w_wdt = wp.tile([128, DIN_SH], BF16, tag="w_wdt")
            w_wo = wp.tile([128, NCT, DM], BF16, tag="w_wo")
            cst = wp.tile([128, _NC], F32, tag="cst")
            e32_sb = wp.tile([32, 128], BF16, tag="e32")
            eb_sb = wp.tile([64, 16, 128], BF16, tag="eb")

            ec_sb = wp.tile([128, 16, 128], BF16, tag="ec")

            nc.sync.dma_start(out=w_wxb[:], in_=wxbT.rearrange("(kt p) c -> p kt c", p=128))
            nc.sync.dma_start(out=w_wz[:], in_=wzT.rearrange("(kt p) c -> p kt c", p=128))
            nc.sync.dma_start(out=w_wc[:], in_=wcT.rearrange("(kt p) c -> p kt c", p=128))
            nc.sync.dma_start(out=w_wdd[:], in_=wddT.rearrange("(kt p) c -> p kt c", p=128))
            nc.sync.dma_start(out=w_wdt[:], in_=wdtT[:])
            nc.sync.dma_start(out=w_wo[:], in_=woutT.rearrange("(ki p) m -> p ki m", p=128))
            nc.sync.dma_start(out=cst[:], in_=consts[:])
            nc.sync.dma_start(out=e32_sb[:], in_=e32[:])
            nc.sync.dma_start(out=eb_sb[:], in_=ebs.rearrange("p (s c) -> p s c", s=16))
            nc.sync.dma_start(out=ec_sb[:], in_=ecs.rearrange("p (s c) -> p s c", s=16))

            # persistent state
            carry = wp.tile([128, DIN_SH // 128 * 16], F32, tag="carry")  # (128, 128)
            xbprev = wp.tile([128, NCT * 3], BF16, tag="xbprev")
            nc.vector.memset(carry[:], 0.0)

            hsT_r = hsT.rearrange("(kt p) t -> p kt t", p=128)

            def col(c0, i, n=1):
                return cst[:, c0 + i:c0 + i + n]

            for k in range(NT):
                hs = hsp.tile([128, 16, T], BF16, tag="hs")
                nc.sync.dma_start(out=hs[:], in_=hsT_r[:, :, k * T:(k + 1) * T])

                # dtlow = hs @ Wdt_down.T  -> (128, T)
                ps_dl = psA.tile([128, T], F32, tag="psA")
                for kt in range(16):
                    nc.tensor.matmul(ps_dl[:], lhsT=w_wdd[:, kt, :], rhs=hs[:, kt, :],
                                     start=(kt == 0), stop=(kt == 15))
                dl = sp.tile([128, T], BF16, tag="dl")
                nc.scalar.copy(out=dl[:], in_=ps_dl[:])

                yf = big.tile([128, NCT, T], BF16, tag="yf")

                for i in range(NCT):
                    # ---- x+B combined projection (64 rows: x 0:32, B 32:64)
                    ps_x = psA.tile([64, T], F32, tag="psA")
                    for kt in range(16):
                        nc.tensor.matmul(ps_x[:], lhsT=w_wxb[:, kt, i * 64:(i + 1) * 64],
                                         rhs=hs[:, kt, :], start=(kt == 0), stop=(kt == 15))
                    x_sb = sq.tile([32, T], BF16, tag="x_sb")
                    nc.scalar.activation(x_sb[:], ps_x[0:32, :], AF.Identity,
                                         bias=cst[0:32, _BX0 + i:_BX0 + i + 1], scale=1.0)
                    b_sb64 = sq.tile([64, T], BF16, tag="b_sb64")
                    nc.scalar.copy(out=b_sb64[32:64, :], in_=ps_x[32:64, :])
                    ps_xb = psA.tile([128, T], F32, tag="psA")
                    nc.tensor.matmul(ps_xb[:], lhsT=e32_sb[:], rhs=x_sb[:], start=True, stop=True)
                    xb = sp.tile([128, T + 3], BF16, tag="xb")
                    if k == 0:
                        nc.vector.memset(xb[:, 0:3], 0.0)
                    else:
                        nc.vector.tensor_copy(xb[:, 0:3], xbprev[:, i * 3:(i + 1) * 3])
                    nc.scalar.copy(out=xb[:, 3:T + 3], in_=ps_xb[:])
                    nc.vector.tensor_copy(xbprev[:, i * 3:(i + 1) * 3], xb[:, T:T + 3])

                    # ---- causal depthwise conv (4 taps) on DVE
                    cv_a = sp.tile([128, T], BF16, tag="cv_a")
                    cv_b = sp.tile([128, T], BF16, tag="cv_b")
                    nc.vector.tensor_scalar(out=cv_a[:], in0=xb[:, 0:T],
                                            scalar1=col(_W40, i * 4 + 0), scalar2=None, op0=OP.mult)
                    nc.vector.scalar_tensor_tensor(out=cv_b[:], in0=xb[:, 1:T + 1],
                                                   scalar=col(_W40, i * 4 + 1), in1=cv_a[:],
                                                   op0=OP.mult, op1=OP.add)
                    nc.vector.scalar_tensor_tensor(out=cv_a[:], in0=xb[:, 2:T + 2],
                                                   scalar=col(_W40, i * 4 + 2), in1=cv_b[:],
                                                   op0=OP.mult, op1=OP.add)
                    nc.vector.scalar_tensor_tensor(out=cv_b[:], in0=xb[:, 3:T + 3],
                                                   scalar=col(_W40, i * 4 + 3), in1=cv_a[:],
                                                   op0=OP.mult, op1=OP.add)
                    # u = silu(cv + cb); sigmoid via exp/ln only (one ACT table set):
                    # sg = exp(-ln(1 + exp(-(cv+cb))))
                    e1u = sq.tile([128, T], BF16, tag="e1")
                    nc.scalar.activation(e1u[:], cv_b[:], AF.Exp,
                                         bias=col(_CBN, i), scale=-1.0)
                    spu = sq.tile([128, T], BF16, tag="spl")
                    nc.scalar.activation(spu[:], e1u[:], AF.Ln, bias=1.0, scale=1.0)
                    sgu = sp.tile([128, T], BF16, tag="sgu")
                    nc.scalar.activation(sgu[:], spu[:], AF.Exp, bias=0.0, scale=-1.0)
                    u = sp.tile([128, T], BF16, tag="u")
                    nc.vector.scalar_tensor_tensor(out=u[:], in0=cv_b[:], scalar=col(_CB0, i),
                                                   in1=sgu[:], op0=OP.add, op1=OP.mult)

                    # ---- z projection + silu
                    ps_z = psA.tile([128, T], F32, tag="psA")
                    for kt in range(16):
                        nc.tensor.matmul(ps_z[:], lhsT=w_wz[:, kt, i * 128:(i + 1) * 128],
                                         rhs=hs[:, kt, :], start=(kt == 0), stop=(kt == 15))
                    e1z = sq.tile([128, T], BF16, tag="e1")
                    nc.scalar.activation(e1z[:], ps_z[:], AF.Exp,
                                         bias=col(_BZN, i), scale=-1.0)
                    spz = sq.tile([128, T], BF16, tag="spl")
                    nc.scalar.activation(spz[:], e1z[:], AF.Ln, bias=1.0, scale=1.0)
                    sgz = sp.tile([128, T], BF16, tag="sgz")
                    nc.scalar.activation(sgz[:], spz[:], AF.Exp, bias=0.0, scale=-1.0)
                    sz = sp.tile([128, T], BF16, tag="sz")
                    nc.vector.scalar_tensor_tensor(out=sz[:], in0=ps_z[:], scalar=col(_BZ0, i),
                                                   in1=sgz[:], op0=OP.add, op1=OP.mult)

                    # ---- delta = softplus(dt_raw + 2*bdt)
                    ps_d = psA.tile([128, T], F32, tag="psA")
                    nc.tensor.matmul(ps_d[:], lhsT=w_wdt[:, i * 128:(i + 1) * 128],
                                     rhs=dl[:], start=True, stop=True)
                    et = sq.tile([128, T], BF16, tag="et")
                    nc.scalar.activation(et[:], ps_d[:], AF.Exp,
                                         bias=col(_BDT2, i), scale=1.0)
                    dlt = sp.tile([128, T], BF16, tag="dlt")
                    nc.scalar.activation(dlt[:], et[:], AF.Ln, bias=1.0, scale=1.0)
                    du = sp.tile([128, T], BF16, tag="du")
                    nc.vector.tensor_tensor(out=du[:], in0=dlt[:], in1=u[:], op=OP.mult)

                    # ---- C projection (128 rows: (g,s))
                    ps_c = psA.tile([128, T], F32, tag="psA")
                    for kt in range(16):
                        nc.tensor.matmul(ps_c[:], lhsT=w_wc[:, kt, i * 128:(i + 1) * 128],
                                         rhs=hs[:, kt, :], start=(kt == 0), stop=(kt == 15))
                    c_sb = sp.tile([128, T], BF16, tag="c_sb")
                    nc.scalar.copy(out=c_sb[:], in_=ps_c[:])

                    # ---- selective scan over 16 states
                    h = big.tile([128, 16, T], BF16, tag="h")
                    for s in range(16):
                        dA = sp3.tile([128, T], BF16, tag="dA")
                        nc.scalar.activation(dA[:], dlt[:], AF.Exp, bias=0.0,
                                             scale=col(_A0, i * 16 + s))
                        bc = psT.tile([128, 2, T], F32, tag="bc")
                        bb = bc[:, 0, :]
                        cb = bc[:, 1, :]
                        nc.tensor.matmul(bb, lhsT=eb_sb[32:64, s, :], rhs=b_sb64[32:64, :],
                                         start=True, stop=True)
                        nc.tensor.matmul(cb, lhsT=ec_sb[:, s, :], rhs=c_sb[:],
                                         start=True, stop=True)
                        b_in = sp3.tile([128, T], BF16, tag="b_in")
                        nc.vector.tensor_tensor(out=b_in[:], in0=du[:], in1=bb, op=OP.mult)
                        hsl = h[:, s, :]
                        nc.vector.tensor_tensor_scan(out=hsl, data0=dA[:], data1=b_in[:],
                                                     initial=carry[:, i * 16 + s:i * 16 + s + 1],
                                                     op0=OP.mult, op1=OP.add)
                        if s % 2 == 0:
                            m_e = sp.tile([128, T], BF16, tag="m_e")
                            nc.vector.tensor_tensor(out=m_e[:], in0=hsl, in1=cb, op=OP.mult)
                        else:
                            m_o = sp.tile([128, T], BF16, tag="m_o")
                            nc.vector.tensor_tensor(out=m_o[:], in0=hsl, in1=cb, op=OP.mult)
                            # save scan-carry for this pair, then reuse h[:, s-1, :]
                            nc.vector.tensor_copy(carry[:, i * 16 + s - 1:i * 16 + s + 1],
                                                  h[:, s - 1:s + 1, T - 1:T])
                            nc.gpsimd.tensor_tensor(out=h[:, s - 1, :], in0=m_e[:],
                                                    in1=m_o[:], op=OP.add)
                    # reduce pair sums (slots 0,2,..,14) in-place in h
                    nc.gpsimd.tensor_tensor(out=h[:, 1:16:4, :], in0=h[:, 0:16:4, :],
                                            in1=h[:, 2:16:4, :], op=OP.add)
                    nc.vector.tensor_tensor(out=h[:, 3:16:8, :], in0=h[:, 1:16:8, :],
                                            in1=h[:, 5:16:8, :], op=OP.add)
                    y = sp.tile([128, T], BF16, tag="y")
                    nc.vector.tensor_tensor(out=y[:], in0=h[:, 3, :], in1=h[:, 11, :],
                                            op=OP.add)
                    # yf = (y + u*D) * silu(z)
                    t_yf = sp.tile([128, T], BF16, tag="t_yf")
                    nc.vector.scalar_tensor_tensor(out=t_yf[:], in0=u[:], scalar=col(_D0, i),
                                                   in1=y[:], op0=OP.mult, op1=OP.add)
                    nc.vector.tensor_tensor(out=yf[:, i, :], in0=t_yf[:], in1=sz[:], op=OP.mult)

                # ---- output projection partials: out[dm, t] = sum_i Wout_i.T @ yf_i
                for half in range(2):
                    for dmt in range(16):
                        ps_o = psO.tile([128, TH], F32, tag="psO")
                        for i in range(NCT):
                            nc.tensor.matmul(ps_o[:],
                                             lhsT=w_wo[:, i, dmt * 128:(dmt + 1) * 128],
                                             rhs=yf[:, i, half * TH:(half + 1) * TH],
                                             start=(i == 0), stop=(i == NCT - 1))
                        o_sb = sp.tile([128, TH], BF16, tag="o_sb")
                        nc.scalar.copy(out=o_sb[:], in_=ps_o[:])
                        nc.sync.dma_start(
                            out=outp[dmt * 128:(dmt + 1) * 128,
                                     k * T + half * TH:k * T + (half + 1) * TH],
                            in_=o_sb[:])
    _legalize_waits(nc)
    return nc


def _legalize_waits(nc):
    """This walrus build allows one sync-wait per instruction; split extras
    into standalone EventSemaphore waits on the same engine."""
    from concourse import mybir
    n = 0
    for fn in nc.m.functions:
        for blk in fn.blocks:
            newi = []
            for ins in blk.instructions:
                si = ins.sync_info
                if si is not None and si.on_wait is not None and len(si.on_wait) > 1:
                    for w in si.on_wait[:-1]:
                        ev = mybir.InstEventSemaphore(
                            name=f"W-{n}", ins=[], outs=[],
                            sync_info=mybir.SyncInfo(on_wait=[w], on_update=[]))
                        ev.engine = ins.engine
                        newi.append(ev)
                        n += 1
                    si.on_wait = [si.on_wait[-1]]
                newi.append(ins)
            blk.instructions = newi
    return n


def _prep_inputs(inputs, L):
    hs = np.asarray(inputs["hidden_states"], np.float32)
    Wx = np.asarray(inputs["Wx"], np.float32)
    bx = np.asarray(inputs["bx"], np.float32)
    Wz = np.asarray(inputs["Wz"], np.float32)
    bz = np.asarray(inputs["bz"], np.float32)
    conv_w = np.asarray(inputs["conv_w"], np.float32)
    conv_b = np.asarray(inputs["conv_b"], np.float32)
    WB = np.asarray(inputs["WB"], np.float32)
    WC = np.asarray(inputs["WC"], np.float32)
    Wdd = np.asarray(inputs["Wdt_down"], np.float32)
    Wdt = np.asarray(inputs["Wdt"], np.float32)
    bdt = np.asarray(inputs["bdt"], np.float32)
    A = -np.exp(np.asarray(inputs["A_log"], np.float32))
    D = np.asarray(inputs["D"], np.float32)
    Wout = np.asarray(inputs["Wout"], np.float32)

    e32 = np.zeros((32, 128), BF)
    for c in range(128):
        e32[(c // 64) * 16 + (c % 16), c] = 1
    ebs = np.zeros((64, 16, 128), BF)
    ecs = np.zeros((128, 16, 128), BF)
    for s in range(DS):
        for c in range(128):
            ebs[32 + (c // 64) * 16 + s, s, c] = 1
            ecs[(c // 16) * 16 + s, s, c] = 1
    ebs = ebs.reshape(64, 16 * 128)
    ecs = ecs.reshape(128, 16 * 128)

    in_maps = []
    for core in range(NCORES):
        bi, ci = core // NCH, core % NCH
        ch0 = ci * DIN_SH
        cs = slice(ch0, ch0 + DIN_SH)
        xs = slice(ci * DXB_SH, (ci + 1) * DXB_SH)
        consts = np.zeros((128, _NC), np.float32)
        for i in range(NCT):
            rows = slice(i * 128, (i + 1) * 128)
            consts[:, _A0 + i * 16:_A0 + (i + 1) * 16] = A[cs][rows]
            consts[:, _D0 + i] = D[cs][rows]
            consts[:, _CB0 + i] = conv_b[cs][rows]
            consts[:, _CBN + i] = -conv_b[cs][rows]
            consts[:, _BDT2 + i] = 2.0 * bdt[cs][rows]
            consts[:, _BZ0 + i] = bz[cs][rows]
            consts[:, _BZN + i] = -bz[cs][rows]
            for j in range(DC):
                consts[:, _W40 + i * 4 + j] = conv_w[cs, 0, j][rows]
            consts[0:32, _BX0 + i] = bx[xs][i * 32:(i + 1) * 32]
        # pack x and B projection weights: per ctile 64 cols = [x 32 | B 32]
        wxb = np.zeros((2 * DXB_SH, DM), np.float32)
        for i in range(NCT):
            wxb[i * 64:i * 64 + 32] = Wx[xs][i * 32:(i + 1) * 32]
            wxb[i * 64 + 32:i * 64 + 64] = WB[xs][i * 32:(i + 1) * 32]
        in_maps.append({
            "hsT": np.ascontiguousarray(hs[bi, :L].T).astype(BF),
            "wxbT": np.ascontiguousarray(wxb.T).astype(BF),
            "wzT": np.ascontiguousarray(Wz[cs].T).astype(BF),
            "wcT": np.ascontiguousarray(WC[cs].T).astype(BF),
            "wddT": np.ascontiguousarray(Wdd.T).astype(BF),
            "wdtT": np.ascontiguousarray(Wdt[cs].T).astype(BF),
            "woutT": np.ascontiguousarray(Wout[:, cs].T).astype(BF),
            "consts": consts,
            "e32": e32,
            "ebs": ebs,
            "ecs": ecs,
        })
    return in_maps


def _install_profile_hook():
    """Make run_bass_kernel_spmd(trace=True) work: provide the
    antenv.axon_hooks registry the boot script looks for, backed by the
    ctypes NTFF start/stop calls into libaxon_pjrt.so."""
    import contextlib
    import ctypes
    import types

    import concourse.bass_utils as bu
    bu.upload_artifacts = lambda d: d  # no bucket in this container

    if "antenv.axon_hooks" not in sys.modules:
        mod = types.ModuleType("antenv.axon_hooks")
        _store = {}
        mod.set_axon_ntff_profile_hook = lambda h: _store.__setitem__("h", h)
        mod.get_axon_ntff_profile_hook = lambda: _store.get("h")
        sys.modules["antenv.axon_hooks"] = mod
        import antenv
        antenv.axon_hooks = mod

    from antenv.axon_hooks import get_axon_ntff_profile_hook, set_axon_ntff_profile_hook
    if get_axon_ntff_profile_hook() is not None:
        return
    lib = ctypes.CDLL("/opt/axon/libaxon_pjrt.so")
    if not hasattr(lib, "axon_start_nrt_profile"):
        return
    lib.axon_start_nrt_profile.argtypes = [ctypes.POINTER(ctypes.c_int64), ctypes.c_size_t]
    lib.axon_start_nrt_profile.restype = ctypes.c_int64
    lib.axon_stop_nrt_profile.argtypes = [ctypes.c_char_p]
    lib.axon_stop_nrt_profile.restype = ctypes.c_int64

    @contextlib.contextmanager
    def _hook(output_dir, device_ids):
        import jax
        jax.devices()
        if device_ids:
            ids = (ctypes.c_int64 * len(device_ids))(*device_ids)
            rc = lib.axon_start_nrt_profile(ids, len(device_ids))
        else:
            rc = lib.axon_start_nrt_profile(None, 0)
        if rc != 0:
            raise RuntimeError(f"axon_start_nrt_profile rc={rc}")
        try:
            yield
        finally:
            n = lib.axon_stop_nrt_profile(str(output_dir).encode())
            print(f"profile: {n} file(s) written to {output_dir}")

    set_axon_ntff_profile_hook(_hook)


def kernel(**inputs):
    from concourse.bass_utils import run_bass_kernel_spmd

    L, T = L_, 512
    nc = _build(L, T)
    in_maps = _prep_inputs(inputs, L)
    trace = bool(int(os.environ.get("MAMBA_PROFILE", "0")))
    tmpdir = None
    if trace:
        import tempfile
        _install_profile_hook()
        tmpdir = tempfile.mkdtemp(prefix="mamba_trace_")
        kernel.last_trace_dir = tmpdir
    res = run_bass_kernel_spmd(nc, in_maps, core_ids=list(range(NCORES)), trace=trace,
                               tmpdir=tmpdir)
    if trace:
        kernel.last_exec_time_ns = res.exec_time_ns
        kernel.last_profile = res
    bout = np.asarray(inputs["bout"], np.float32)
    out = np.zeros((B_, L_, DM), np.float32)
    for bi in range(B_):
        acc = np.zeros((DM, L_), np.float32)
        for ci in range(NCH):
            acc += np.asarray(res.results[bi * NCH + ci]["outp"], np.float32)
        out[bi] = acc.T + bout[None, :]
    return out
